# revision 12
# baseline (speedup 1.0000x reference)
"""Trainium2 Bass kernel for BertSelfAttentionDistance.

Problem (per batch b, B=8, S=2048, H=1024, NH=1, DT=64):
    q = hs @ Wq.T + bq ; k = hs @ Wk.T + bk ; v = hs @ Wv.T + bv
    scores = q @ k.T
    wdd    = q @ dist_emb.T                       # [S, DT]
    bias   = take_along(wdd, rel, axis=-1) * (rel == 1)
    out    = softmax((scores + bias)/sqrt(H) + attn_mask) @ v

Key simplifications:
1. Because the gathered value is multiplied by (rel == 1), only
   wdd[:, 1] = q @ dist_emb[1] survives:
       bias[i, j] = (rel[i, j] == 1) * (q[i] . dist_emb[1])
   so the [S, S] gather is never needed — just a compare and broadcast.
   The compare itself moves to the host: the kernel receives the 0/1
   mask as uint8 (4x less DMA than the int32 relation tensor).
2. q and k are never needed individually — only scores and w1:
       scores0 = hs @ (Wq.T @ Wk) @ hs.T = G @ hs.T,   G = hs @ M
       w1      = hs @ (Wq.T @ dist_emb[1]) (+ bq . dist_emb[1])
   M = Wq.T @ Wk and m1 = Wq.T @ d1 are folded on the host (weight-only,
   O(H^2) preprocessing); this removes one full [S,H]x[H,H] projection
   per core and the hs tile doubles as the scores stationary.
   Bias terms: q0.bk and bq.bk are constant per query row -> softmax-
   invariant, dropped exactly. bq.k0[k] varies per key: handled by a
   conditionally-compiled kb path (has_bq) feeding the exp bias; the
   benchmark has bq = 0 so the lean variant is used.

Sharding: pure data-parallel, one batch per NeuronCore (8 batches, 8 cores).

Per-core layout (ST = transposed-scores layout, keys on partitions):
    GT[d, s]  (via M stationary / hsT moving; stored fp8e4)
    ST[j, q] = sum_d hsT[d, j] * GT[d, q]         (keys j on partitions)
    E[j, q]  = exp(ST/32 + mask*w1[q]/32 + am[j])   (unnormalized)
    denom[q] = ones[1,j] @ E    (PE reduction over partitions, 4x col-tiled)
    ctxT[d, q] = (sum_j (V[j, d]+bv[d]) * E[j, q]) * (1/denom[q])

Mixed precision: the scores contraction runs entirely in fp8e4 DoubleRow
pairs (2x PE throughput); G, V and PV stay bf16 (quantizing the PV pair
or either projection pushes max-rel past the 2e-2 gate: E-quantization
noise doesn't cancel, and projection-weight quantization error does not
average over keys).  Output is written bf16 (adds <5e-5 rel).  Measured
end-to-end max-rel error ~1.89e-2 of output scale (deterministic for the
benchmark inputs) vs the 2e-2 gate.

v bias note: softmax rows sum to 1, so ctx = P @ (V0 + 1*bv) = P@V0 + bv;
with unnormalized E: (E@(V0+1*bv)) / denom = ctx0 + bv exactly.

Startup: the first m column-block goes on the Sync DGE queue and the
four 2-tile pieces of hs chunk 0 on the Scalar (Activation) DGE queue,
so the ~0.6us/descriptor serial issue cost is split across two engines
and the G loop starts ~5us earlier than a single-queue issue stream.
Dummy matmuls cover the remaining issue+landing latency.
"""

import sys

sys.path.insert(0, "/opt/trn_rl_repo")

from contextlib import ExitStack

import ml_dtypes
import numpy as np

import concourse.bass as bass
import concourse.tile as tile
from concourse import bacc, mybir
from concourse._compat import with_exitstack
from concourse.bass_utils import run_bass_kernel_spmd

B, S, H, DT = 8, 2048, 1024, 64
NCORES = 8
P = 128
CHUNK = 512  # q-chunk = one fp32 PSUM bank
SCALE = 1.0 / 32.0  # 1/sqrt(H)
NDUMMY = 14

BF16 = mybir.dt.bfloat16
F8 = mybir.dt.float8e4
F32 = mybir.dt.float32
U8 = mybir.dt.uint8
AF = mybir.ActivationFunctionType
ALU = mybir.AluOpType
DR = mybir.MatmulPerfMode.DoubleRow
# k-tiles of the scores contraction computed in bf16; the remaining
# HT - FP8_K0 run as fp8e4 DoubleRow pairs at 2x.  0 = all fp8
# (max-rel ~1.89e-2 vs the 2e-2 gate); fallback 2 -> ~1.63e-2.
FP8_K0 = 0

NPBF16 = ml_dtypes.bfloat16


def _bcast_ap(src_row):
    """Partition-broadcast AP: read one [1, N] row as [128, N]."""
    return bass.AP(
        tensor=src_row.tensor,
        offset=src_row.offset,
        ap=[[0, P], list(src_row.ap[-1])],
    )


@with_exitstack
def _attn_kernel(
    ctx: ExitStack, tc: tile.TileContext, outs, ins, s=S, has_bq=False, has_bv=True
):
    nc = tc.nc
    JT = s // P  # key tiles along sequence
    NCH = s // CHUNK  # query chunks
    HT = H // P  # 8
    HC = H // CHUNK  # 2

    hsT = ins["hsT"].rearrange("(t p) s -> p t s", p=P)  # [128, HT, s]
    mT = ins["m"].rearrange("(t p) o -> p t o", p=P)  # [128, HT, H]
    wvT = ins["wvT"].rearrange("(t p) o -> p t o", p=P)
    relT = ins["relT"].rearrange("(t p) q -> p t q", p=P)  # [128, JT, s] u8
    ctxT = outs["ctxT"].rearrange("(t p) q -> p t q", p=P)  # [128, HT, s]

    consts = ctx.enter_context(tc.tile_pool(name="consts", bufs=1))
    qkv = ctx.enter_context(tc.tile_pool(name="qkv", bufs=1))

    # persistent operands for the attention stage.  hs_sb stays resident:
    # it is the moving operand of G/w1 and the stationary of the V
    # projection.  The scores contraction runs on the fp8 copies
    # (G8/hs8) as DoubleRow pairs, plus FP8_K0 bf16 tiles (GTb/hs_sb).
    NF8 = HT - FP8_K0  # fp8 k-tiles
    hs_sb = qkv.tile([P, HT, s], BF16)
    if FP8_K0:
        GTb = qkv.tile([P, FP8_K0, s], BF16)
    G8 = qkv.tile([P, NF8, s], F8)
    hs8 = qkv.tile([P, NF8, s], F8)
    V = qkv.tile([P, JT, H], BF16)
    W1B = qkv.tile([P, NCH, CHUNK], BF16)  # (q . d1)/32, partition-broadcast
    # 0/1 mask in bf16, double-buffered two q-chunks at a time (32KB/
    # partition total).  bf16 keeps the biasT multiply on the DVE 2x
    # 16-bit path (the u8 variant ran 1x and 3-5x slower); prefetching a
    # whole chunk ~44us ahead keeps mask DMA off the score critical path.
    mask_t = [qkv.tile([P, JT, CHUNK], BF16, name=f"mask{i}") for i in range(2)]

    # small per-partition constants (issued after the critical startup
    # wave below — each dma_start costs ~0.6us of issue time on its
    # engine's queue, and these aren't needed until after the G loop)
    m1_s = consts.tile([P, HT], BF16)
    am_s = consts.tile([P, JT], F32)
    if has_bv:
        bvB = consts.tile([P, H], BF16)
    ones_bf = consts.tile([P, 1], BF16)
    if has_bq:
        mb_s = consts.tile([P, HT], BF16)
        c_s = consts.tile([1, 1], F32)

    # scores-phase PSUM pools opened BEFORE stage A's pool so they land in
    # banks stage A never touches — chunk-0 scores can start while stage A
    # epilogues drain.
    ps_score = ctx.enter_context(tc.tile_pool(name="ps_s", bufs=3, space="PSUM"))
    ps_vec = ctx.enter_context(tc.tile_pool(name="ps_vec", bufs=1, space="PSUM"))
    dram_scratch = ctx.enter_context(
        tc.tile_pool(name="dscratch", bufs=3 if has_bq else 2, space="DRAM")
    )
    # bias pool outside the stage-A region so chunk-0's biasT writes don't
    # WAR-wait on stage-A's hs/w tiles.
    biaspool = ctx.enter_context(tc.tile_pool(name="bias", bufs=3))

    # ---- stage A: G projection, w1, V ----
    with (
        tc.tile_pool(name="stage_a", bufs=1) as sa,
        tc.tile_pool(name="psum_a", bufs=3, space="PSUM") as psa,
    ):
        m_sb = sa.tile([P, HT, H], BF16)
        wv_sb = sa.tile([P, HT, H], BF16)
        # Critical startup wave, split across the two HWDGE issue queues:
        # Sync takes M's first column block then the remaining blocks;
        # Scalar takes hs chunk 0 as four 2-tile descriptors.  The G loop
        # (c outer, ot inner) consumes hs chunk 0 it-tile by it-tile as
        # the pieces land, then one fresh 1MB hs chunk per c-pass.
        # Sync streams all M column blocks (the ot-passes consume them in
        # order); Scalar streams hs chunk 0 as four 2-tile pieces (the
        # it-accumulation tracks the landing stream).
        for ob in range(HT):
            nc.sync.dma_start(
                out=m_sb[:, :, ob * P : (ob + 1) * P],
                in_=mT[:, :, ob * P : (ob + 1) * P],
            )
        for tt in range(0, HT, 2):
            nc.scalar.dma_start(
                out=hs_sb[:, tt : tt + 2, 0:CHUNK],
                in_=hsT[:, tt : tt + 2, 0:CHUNK],
            )
        # consts ride the scalar queue behind the critical wave
        nc.scalar.dma_start(out=m1_s, in_=ins["m1"])
        nc.scalar.dma_start(out=am_s, in_=ins["am"])
        if has_bv:
            nc.scalar.dma_start(out=bvB, in_=_bcast_ap(ins["bv"]))
        nc.vector.memset(ones_bf, 1.0)
        if has_bq:
            nc.scalar.dma_start(out=mb_s, in_=ins["mb"])
            nc.scalar.dma_start(out=c_s, in_=ins["c"])
        for c in range(1, NCH):
            nc.sync.dma_start(
                out=hs_sb[:, :, c * CHUNK : (c + 1) * CHUNK],
                in_=hsT[:, :, c * CHUNK : (c + 1) * CHUNK],
            )
        nc.sync.dma_start(out=wv_sb, in_=wvT)
        # masks for q-chunks 0/1 (chunks 2/3 stream in during stage B)
        for i in range(2):
            nc.sync.dma_start(
                out=mask_t[i], in_=relT[:, :, i * CHUNK : (i + 1) * CHUNK]
            )

        # Dummy matmuls: keep the PE busy (and the HAM clock-gate warm)
        # while the first m/hs tiles stream in. Zero data, never consumed.
        dummy_src = sa.tile([P, 256], BF16)
        nc.vector.memset(dummy_src, 0.0)
        dummy_ps = ps_vec.tile([P, 256], F32, tag="vec1", name="dummy_ps")
        for _ in range(NDUMMY):
            nc.tensor.matmul(
                dummy_ps,
                dummy_src[:, 0:P],
                dummy_src,
                start=True,
                stop=True,
                skip_group_check=True,
            )

        # GT[o, s] = sum_i M[i, o] * hsT[i, s].  c outer / ot inner: each
        # c-pass re-reads the already-resident M and consumes exactly one
        # fresh hs chunk, so the PE never outruns the startup DMA stream.
        for c in range(NCH):
            for ot in range(HT):
                ps_g = psa.tile([P, CHUNK], F32, tag="pa", name="pa_g")
                for it in range(HT):
                    nc.tensor.matmul(
                        ps_g,
                        m_sb[:, it, ot * P : (ot + 1) * P],
                        hs_sb[:, it, c * CHUNK : (c + 1) * CHUNK],
                        start=(it == 0),
                        stop=(it == HT - 1),
                    )
                tgt = (
                    GTb[:, ot, c * CHUNK : (c + 1) * CHUNK]
                    if ot < FP8_K0
                    else G8[:, ot - FP8_K0, c * CHUNK : (c + 1) * CHUNK]
                )
                nc.scalar.activation(tgt, ps_g, AF.Identity, bias=0.0)

        # W1B precompute: w1[q]/32 = hs[q] . m1/32, all chunks, col-tiled 4x.
        # Emitted right after G so the DRAM-broadcast roundtrip completes
        # long before chunk 0 needs it.
        w1p = ps_vec.tile([P, CHUNK], F32, tag="vec1")
        for it in range(HT):
            for c in range(NCH):
                nc.tensor.matmul(
                    w1p[32 * c : 32 * c + 1, :],
                    m1_s[:, it : it + 1],
                    hs_sb[:, it, c * CHUNK : (c + 1) * CHUNK],
                    start=(it == 0),
                    stop=(it == HT - 1),
                    tile_position=(0, 32 * c),
                    skip_group_check=True,
                )
        w1rows = consts.tile([1, NCH, CHUNK], BF16)
        w1d = dram_scratch.tile([1, NCH, CHUNK], BF16)
        for c in range(NCH):
            if has_bq:
                # w1_full/32 = hs.m1/32 + (bq.d1)/32
                nc.scalar.activation(
                    w1rows[:, c, :],
                    w1p[32 * c : 32 * c + 1, :],
                    AF.Identity,
                    bias=c_s[0:1, 0:1],
                )
            else:
                nc.vector.tensor_copy(w1rows[:, c, :], w1p[32 * c : 32 * c + 1, :])
            nc.sync.dma_start(out=w1d[:, c, :], in_=w1rows[:, c, :])
            nc.sync.dma_start(out=W1B[:, c, :], in_=_bcast_ap(w1d[:, c, :]))

        if has_bq:
            # kb[k]/32 = hs[k] . (Wk.T bq)/32, added to the per-key exp bias.
            kbp = ps_vec.tile([P, CHUNK], F32, tag="vec1")
            for it in range(HT):
                for c in range(NCH):
                    nc.tensor.matmul(
                        kbp[32 * c : 32 * c + 1, :],
                        mb_s[:, it : it + 1],
                        hs_sb[:, it, c * CHUNK : (c + 1) * CHUNK],
                        start=(it == 0),
                        stop=(it == HT - 1),
                        tile_position=(0, 32 * c),
                        skip_group_check=True,
                    )
            kbrow = consts.tile([1, NCH, CHUNK], F32)
            for c in range(NCH):
                nc.vector.tensor_copy(kbrow[:, c, :], kbp[32 * c : 32 * c + 1, :])
            kbd = dram_scratch.tile([1, NCH, CHUNK], F32)
            nc.sync.dma_start(out=kbd, in_=kbrow)
            kb_s = consts.tile([P, JT], F32)
            nc.sync.dma_start(
                out=kb_s,
                in_=bass.AP(tensor=kbd.tensor, offset=kbd.offset, ap=[[1, P], [P, JT]]),
            )
            am_eff = consts.tile([P, JT], F32)
            nc.vector.tensor_tensor(am_eff, am_s, kb_s, op=ALU.add)
            am_x = am_eff
        else:
            am_x = am_s

        # fp8 copies of the hs k-tiles used by the DoubleRow score matmuls
        # (scalar engine; overlaps the V matmuls below)
        for i in range(NF8):
            nc.scalar.activation(
                hs8[:, i, :], hs_sb[:, i + FP8_K0, :], AF.Identity, bias=0.0
            )

        # V[j, o] = sum_i hsT[i, j] * WvT[i, o] + bv[o].  V last: it has no
        # chunk-0 consumers until PV, so its matmuls give the scheduler PE
        # filler while chunk-0's softmax pipeline warms up.
        for jt in range(JT):
            pss = [
                psa.tile([P, CHUNK], F32, tag="pa", name=f"pav_{i}")
                for i in range(HC)
            ]
            for it in range(HT):
                for oc in range(HC):
                    nc.tensor.matmul(
                        pss[oc],
                        hs_sb[:, it, jt * P : (jt + 1) * P],
                        wv_sb[:, it, oc * CHUNK : (oc + 1) * CHUNK],
                        start=(it == 0),
                        stop=(it == HT - 1),
                    )
            for oc in range(HC):
                if has_bv:
                    nc.vector.tensor_tensor(
                        V[:, jt, oc * CHUNK : (oc + 1) * CHUNK],
                        pss[oc],
                        bvB[:, oc * CHUNK : (oc + 1) * CHUNK],
                        op=ALU.add,
                    )
                else:
                    # scalar-engine copy keeps the vector queue clear for
                    # chunk 0's softmax pipeline
                    nc.scalar.activation(
                        V[:, jt, oc * CHUNK : (oc + 1) * CHUNK],
                        pss[oc],
                        AF.Identity,
                        bias=0.0,
                    )

    # ---- stage B pools ----
    epool = ctx.enter_context(tc.tile_pool(name="E", bufs=2))
    esumpool = ctx.enter_context(tc.tile_pool(name="esum", bufs=2))
    recpool = ctx.enter_context(tc.tile_pool(name="rec", bufs=2))
    outpool = ctx.enter_context(tc.tile_pool(name="out", bufs=3))
    ps_pv = ctx.enter_context(tc.tile_pool(name="ps_pv", bufs=4, space="PSUM"))

    # ---- stage B: per query chunk ----
    # PV for chunk c is emitted after the scores/softmax of chunk c+1, so
    # the denom->reciprocal->broadcast chain of chunk c overlaps an entire
    # scores phase and the PE never waits on it.
    deferred_pv = []

    def emit_pv(c, E, recB, last=False):
        cs = slice(c * CHUNK, (c + 1) * CHUNK)
        for dt in range(HT):
            # For the very last output tile, halve the moving width so the
            # trailing (non-overlapped) multiply+DMA epilogue is half-size.
            nsplit = 2 if (last and dt == HT - 1) else 1
            w = CHUNK // nsplit
            for h in range(nsplit):
                ps2 = ps_pv.tile([P, w], F32)
                for jt in range(JT):
                    nc.tensor.matmul(
                        ps2,
                        V[:, jt, dt * P : (dt + 1) * P],
                        E[:, jt, h * w : (h + 1) * w],
                        start=(jt == 0),
                        stop=(jt == JT - 1),
                    )
                ot_t = outpool.tile([P, w], BF16)
                nc.vector.tensor_tensor(
                    ot_t, ps2, recB[:, h * w : (h + 1) * w], op=ALU.mult
                )
                nc.sync.dma_start(
                    out=ctxT[:, dt, c * CHUNK + h * w : c * CHUNK + (h + 1) * w],
                    in_=ot_t,
                )

    for c in range(NCH):
        cs = slice(c * CHUNK, (c + 1) * CHUNK)

        E = epool.tile([P, JT, CHUNK], BF16)
        # per-partition running sum of E on the (otherwise idle) GpSimd
        # engine, staggered one tile behind the exps.  The old 4x
        # col-tiled PE "denom quads" shared XBUSes with the DoubleRow
        # score matmuls (col-tiling needs 8 of 9, DR needs 2) and
        # stretched both; now the PE only sees two plain [128,1,512]
        # matmuls per chunk for the final cross-partition reduce.
        acc = esumpool.tile([P, CHUNK], F32, tag="acc")
        dps = ps_vec.tile([P, CHUNK], F32, tag="vec1")

        for jt in range(JT):
            biasT = biaspool.tile([P, CHUNK], BF16)
            nc.vector.tensor_tensor(
                biasT, mask_t[c % 2][:, jt, :], W1B[:, c, :], op=ALU.mult
            )
            ps = ps_score.tile([P, CHUNK], F32)
            for dt in range(FP8_K0):
                nc.tensor.matmul(
                    ps,
                    hs_sb[:, dt, jt * P : (jt + 1) * P],
                    GTb[:, dt, cs],
                    start=(dt == 0),
                    stop=False,
                )
            for i in range(0, NF8, 2):
                nc.tensor.matmul(
                    ps,
                    hs8[:, i : i + 2, jt * P : (jt + 1) * P],
                    G8[:, i : i + 2, cs],
                    start=(FP8_K0 == 0 and i == 0),
                    stop=(i == NF8 - 2),
                    perf_mode=DR,
                )
            nc.vector.scalar_tensor_tensor(
                ps, ps, SCALE, biasT, op0=ALU.mult, op1=ALU.add
            )
            nc.scalar.activation(E[:, jt, :], ps, AF.Exp, bias=am_x[:, jt : jt + 1])
            if jt == 0:
                nc.gpsimd.tensor_copy(acc, E[:, 0, :])
            else:
                nc.gpsimd.tensor_tensor(acc, acc, E[:, jt, :], op=ALU.add)

        # next-next chunk's mask reuses this chunk's buffer; issue the DMA
        # now (it WAR-waits on the last biasT read above, then has ~40us
        # to land 2MB before chunk c+2 reads it)
        if c + 2 < NCH:
            nc.sync.dma_start(
                out=mask_t[c % 2],
                in_=relT[:, :, (c + 2) * CHUNK : (c + 3) * CHUNK],
            )
        # acc -> bf16 hi/lo split (exact to ~2^-17) so the cross-partition
        # reduce runs as two cheap bf16 matmuls instead of one 1/4-rate
        # f32 one.  hi/lo on the (fast) DVE: the gpsimd adds have a whole
        # PV phase of slack, but hi/lo sit right before the PE's denom
        # matmuls and gpsimd's ~2us/op latency stalled the PE there.
        hi = esumpool.tile([P, CHUNK], BF16, tag="hi")
        nc.vector.tensor_copy(hi, acc)
        lo = esumpool.tile([P, CHUNK], BF16, tag="lo")
        nc.vector.tensor_tensor(lo, acc, hi, op=ALU.subtract)
        if deferred_pv:
            emit_pv(*deferred_pv.pop(0))
        nc.tensor.matmul(dps[0:1, :], ones_bf, hi, start=True, stop=False)
        nc.tensor.matmul(dps[0:1, :], ones_bf, lo, start=False, stop=True)

        # denom row -> DRAM -> partition-broadcast -> reciprocal
        dsum = recpool.tile([1, CHUNK], F32, tag="dsum")
        nc.vector.tensor_copy(dsum, dps[0:1, :])
        dsum_d = dram_scratch.tile([1, CHUNK], F32, tag="dsum_d")
        nc.sync.dma_start(out=dsum_d, in_=dsum)
        denB = recpool.tile([P, CHUNK], F32, tag="denB")
        nc.sync.dma_start(out=denB, in_=_bcast_ap(dsum_d))
        recB = recpool.tile([P, CHUNK], F32, tag="recB")
        rscr = recpool.tile([P, CHUNK], F32, tag="rscr")
        nc.vector.reciprocal_approx_accurate(recB, denB, rscr)

        deferred_pv.append((c, E, recB))
    while deferred_pv:
        last = len(deferred_pv) == 1
        emit_pv(*deferred_pv.pop(0), last=last)


def build_program(s=S, has_bq=False, has_bv=True):
    """Build + compile the per-core Bass program."""
    JT = s // P
    HT = H // P
    nc = bacc.Bacc("TRN2", target_bir_lowering=False, debug=False)
    ins = {
        "hsT": nc.dram_tensor("hsT", [H, s], BF16, kind="ExternalInput").ap(),
        "m": nc.dram_tensor("m", [H, H], BF16, kind="ExternalInput").ap(),
        "wvT": nc.dram_tensor("wvT", [H, H], BF16, kind="ExternalInput").ap(),
        "m1": nc.dram_tensor("m1", [P, HT], BF16, kind="ExternalInput").ap(),
        "am": nc.dram_tensor("am", [P, JT], F32, kind="ExternalInput").ap(),
        "relT": nc.dram_tensor("relT", [s, s], BF16, kind="ExternalInput").ap(),
    }
    if has_bv:
        ins["bv"] = nc.dram_tensor("bv", [1, H], BF16, kind="ExternalInput").ap()
    if has_bq:
        ins["mb"] = nc.dram_tensor("mb", [P, HT], BF16, kind="ExternalInput").ap()
        ins["c"] = nc.dram_tensor("c", [1, 1], F32, kind="ExternalInput").ap()
    outs = {
        "ctxT": nc.dram_tensor("ctxT", [H, s], BF16, kind="ExternalOutput").ap(),
    }
    with tile.TileContext(nc) as tc:
        _attn_kernel(tc, outs, ins, s=s, has_bq=has_bq, has_bv=has_bv)
    nc.compile()
    return nc


def make_in_maps(
    hidden_states,
    attention_mask,
    word_word_relation,
    Wq,
    bq,
    Wk,
    bk,
    Wv,
    bv,
    dist_emb,
    s=S,
):
    """Host-side sharding/layout marshalling: one batch per core.

    Weight-only folds (O(H^2), batch-independent): M = Wq.T @ Wk,
    m1 = Wq.T @ dist_emb[1].  bk only enters softmax-invariant terms.
    The relation tensor ships as the uint8 mask (rel == 1).
    """
    HT = H // P
    JT = s // P
    hs = np.asarray(hidden_states, dtype=np.float32)
    am = np.asarray(attention_mask, dtype=np.float32)
    rel = np.asarray(word_word_relation)
    maskb = (rel == 1).astype(NPBF16)
    Wqf = np.asarray(Wq, np.float32)
    Wkf = np.asarray(Wk, np.float32)
    Wvf = np.asarray(Wv, np.float32)
    d1 = np.asarray(dist_emb, np.float32)[1]
    m_h = np.ascontiguousarray((Wqf.T @ Wkf).astype(NPBF16))
    m1_h = np.ascontiguousarray(
        ((Wqf.T @ d1) * SCALE).reshape(HT, P).T.astype(NPBF16)
    )
    wvT = np.ascontiguousarray(Wvf.T.astype(NPBF16))
    bvf = np.asarray(bv, np.float32)
    has_bv = bool(np.any(bvf))
    if has_bv:
        bv_s = np.ascontiguousarray(bvf.astype(NPBF16).reshape(1, H))
    bqf = np.asarray(bq, np.float32)
    has_bq = bool(np.any(bqf))
    if has_bq:
        mb_h = np.ascontiguousarray(
            ((Wkf.T @ bqf) * SCALE).reshape(HT, P).T.astype(NPBF16)
        )
        c_h = np.ascontiguousarray(
            np.array([[float(bqf @ d1) * SCALE]], dtype=np.float32)
        )
    in_maps = []
    for b in range(hs.shape[0]):
        hsT = np.ascontiguousarray(hs[b].T.astype(NPBF16))
        relT = np.ascontiguousarray(maskb[b].T)
        am_s = np.ascontiguousarray(am[b, 0, 0].reshape(JT, P).T)
        im = {
            "hsT": hsT,
            "m": m_h,
            "wvT": wvT,
            "m1": m1_h,
            "am": am_s,
            "relT": relT,
        }
        if has_bv:
            im["bv"] = bv_s
        if has_bq:
            im["mb"] = mb_h
            im["c"] = c_h
        in_maps.append(im)
    return in_maps, has_bq, has_bv


_NC_CACHE = {}


def get_program(s=S, has_bq=False, has_bv=False):
    key = (s, has_bq, has_bv)
    if key not in _NC_CACHE:
        _NC_CACHE[key] = build_program(s, has_bq, has_bv)
    return _NC_CACHE[key]


def run(inputs: dict, trace: bool = False):
    """Run on hardware; returns (output [B,S,H] f32, BassKernelResults)."""
    in_maps, has_bq, has_bv = make_in_maps(**inputs)
    nc = get_program(S, has_bq, has_bv)
    res = run_bass_kernel_spmd(nc, in_maps, list(range(NCORES)), trace=trace)
    out = np.stack(
        [np.ascontiguousarray(r["ctxT"].T) for r in res.results], axis=0
    ).astype(np.float32)
    return out, res


def kernel(**inputs) -> np.ndarray:
    try:
        out, _ = run(inputs, trace=False)
    except Exception:
        # transient device/runtime hiccups have been observed once in a
        # while on back-to-back runs; one retry is cheap insurance
        out, _ = run(inputs, trace=False)
    return out


# revision 16
# speedup vs baseline: 1.1816x; 1.1816x over previous
"""Trainium2 Bass kernel for BertSelfAttentionDistance.

Problem (per batch b, B=8, S=2048, H=1024, NH=1, DT=64):
    q = hs @ Wq.T + bq ; k = hs @ Wk.T + bk ; v = hs @ Wv.T + bv
    scores = q @ k.T
    wdd    = q @ dist_emb.T                       # [S, DT]
    bias   = take_along(wdd, rel, axis=-1) * (rel == 1)
    out    = softmax((scores + bias)/sqrt(H) + attn_mask) @ v

Key simplifications:
1. Because the gathered value is multiplied by (rel == 1), only
   wdd[:, 1] = q @ dist_emb[1] survives:
       bias[i, j] = (rel[i, j] == 1) * (q[i] . dist_emb[1])
   so the [S, S] gather is never needed — just a compare and broadcast.
   The compare itself moves to the host: the kernel receives the 0/1
   mask as uint8 (4x less DMA than the int32 relation tensor).
2. q and k are never needed individually — only scores and w1:
       scores0 = hs @ (Wq.T @ Wk) @ hs.T = G @ hs.T,   G = hs @ M
       w1      = hs @ (Wq.T @ dist_emb[1]) (+ bq . dist_emb[1])
   M = Wq.T @ Wk and m1 = Wq.T @ d1 are folded on the host (weight-only,
   O(H^2) preprocessing); this removes one full [S,H]x[H,H] projection
   per core and the hs tile doubles as the scores stationary.
   Bias terms: q0.bk and bq.bk are constant per query row -> softmax-
   invariant, dropped exactly. bq.k0[k] varies per key: handled by a
   conditionally-compiled kb path (has_bq) feeding the exp bias; the
   benchmark has bq = 0 so the lean variant is used.

Sharding: pure data-parallel, one batch per NeuronCore (8 batches, 8 cores).

Per-core layout (ST = transposed-scores layout, keys on partitions):
    GT[d, s]  (via M stationary / hsT moving; stored fp8e4)
    ST[j, q] = sum_d hsT[d, j] * GT[d, q]         (keys j on partitions)
    E[j, q]  = exp(ST/32 + mask*w1[q]/32 + am[j])   (unnormalized)
    denom[q] = ones[1,j] @ E    (PE reduction over partitions, 4x col-tiled)
    ctxT[d, q] = (sum_j (V[j, d]+bv[d]) * E[j, q]) * (1/denom[q])

Mixed precision: the scores contraction runs entirely in fp8e4 DoubleRow
pairs (2x PE throughput); G, V and PV stay bf16 (quantizing the PV pair
or either projection pushes max-rel past the 2e-2 gate: E-quantization
noise doesn't cancel, and projection-weight quantization error does not
average over keys).  Output is written bf16 (adds <5e-5 rel).  Measured
end-to-end max-rel error ~1.89e-2 of output scale (deterministic for the
benchmark inputs) vs the 2e-2 gate.

v bias note: softmax rows sum to 1, so ctx = P @ (V0 + 1*bv) = P@V0 + bv;
with unnormalized E: (E@(V0+1*bv)) / denom = ctx0 + bv exactly.

Startup: the first m column-block goes on the Sync DGE queue and the
four 2-tile pieces of hs chunk 0 on the Scalar (Activation) DGE queue,
so the ~0.6us/descriptor serial issue cost is split across two engines
and the G loop starts ~5us earlier than a single-queue issue stream.
Dummy matmuls cover the remaining issue+landing latency.
"""

import sys

sys.path.insert(0, "/opt/trn_rl_repo")

from contextlib import ExitStack

import ml_dtypes
import numpy as np

import concourse.bass as bass
import concourse.tile as tile
from concourse import bacc, mybir
from concourse._compat import with_exitstack
from concourse.bass_utils import run_bass_kernel_spmd

B, S, H, DT = 8, 2048, 1024, 64
NCORES = 8
P = 128
CHUNK = 512  # q-chunk = one fp32 PSUM bank
SCALE = 1.0 / 32.0  # 1/sqrt(H)
NDUMMY = 14

BF16 = mybir.dt.bfloat16
F8 = mybir.dt.float8e4
F32 = mybir.dt.float32
U8 = mybir.dt.uint8
AF = mybir.ActivationFunctionType
ALU = mybir.AluOpType
DR = mybir.MatmulPerfMode.DoubleRow
# k-tiles of the scores contraction computed in bf16; the remaining
# HT - FP8_K0 run as fp8e4 DoubleRow pairs at 2x.  0 = all fp8
# (max-rel ~1.89e-2 vs the 2e-2 gate); fallback 2 -> ~1.63e-2.
FP8_K0 = 0

NPBF16 = ml_dtypes.bfloat16


def _bcast_ap(src_row):
    """Partition-broadcast AP: read one [1, N] row as [128, N]."""
    return bass.AP(
        tensor=src_row.tensor,
        offset=src_row.offset,
        ap=[[0, P], list(src_row.ap[-1])],
    )


@with_exitstack
def _attn_kernel(
    ctx: ExitStack, tc: tile.TileContext, outs, ins, s=S, has_bq=False, has_bv=True
):
    nc = tc.nc
    JT = s // P  # key tiles along sequence
    NCH = s // CHUNK  # query chunks
    HT = H // P  # 8
    HC = H // CHUNK  # 2

    hsT = ins["hsT"].rearrange("(t p) s -> p t s", p=P)  # [128, HT, s]
    mT = ins["m"].rearrange("(t p) o -> p t o", p=P)  # [128, HT, H]
    wvT = ins["wvT"].rearrange("(t p) o -> p t o", p=P)
    relT = ins["relT"].rearrange("(t p) q -> p t q", p=P)  # [128, JT, s] u8
    ctxT = outs["ctxT"].rearrange("(t p) q -> p t q", p=P)  # [128, HT, s]

    consts = ctx.enter_context(tc.tile_pool(name="consts", bufs=1))
    qkv = ctx.enter_context(tc.tile_pool(name="qkv", bufs=1))

    # persistent operands for the attention stage.  hs_sb stays resident:
    # it is the moving operand of G/w1 and the stationary of the V
    # projection.  The scores contraction runs on the fp8 copies
    # (G8/hs8) as DoubleRow pairs, plus FP8_K0 bf16 tiles (GTb/hs_sb).
    NF8 = HT - FP8_K0  # fp8 k-tiles
    hs_sb = qkv.tile([P, HT, s], BF16)
    if FP8_K0:
        GTb = qkv.tile([P, FP8_K0, s], BF16)
    G8 = qkv.tile([P, NF8, s], F8)
    hs8 = qkv.tile([P, NF8, s], F8)
    V = qkv.tile([P, JT, H], BF16)
    W1B = qkv.tile([P, NCH, CHUNK], BF16)  # (q . d1)/32, partition-broadcast
    # 0/1 mask in bf16, double-buffered two q-chunks at a time (32KB/
    # partition total).  bf16 keeps the biasT multiply on the DVE 2x
    # 16-bit path (the u8 variant ran 1x and 3-5x slower); prefetching a
    # whole chunk ~44us ahead keeps mask DMA off the score critical path.
    mask_t = [qkv.tile([P, JT, CHUNK], BF16, name=f"mask{i}") for i in range(2)]

    # small per-partition constants (issued after the critical startup
    # wave below — each dma_start costs ~0.6us of issue time on its
    # engine's queue, and these aren't needed until after the G loop)
    m1_s = consts.tile([P, HT], BF16)
    am_s = consts.tile([P, JT], F32)
    if has_bv:
        bvB = consts.tile([P, H], BF16)
    ones_bf = consts.tile([P, 1], BF16)
    if has_bq:
        mb_s = consts.tile([P, HT], BF16)
        c_s = consts.tile([1, 1], F32)

    # scores-phase PSUM pools opened BEFORE stage A's pool so they land in
    # banks stage A never touches — chunk-0 scores can start while stage A
    # epilogues drain.
    ps_score = ctx.enter_context(tc.tile_pool(name="ps_s", bufs=3, space="PSUM"))
    ps_vec = ctx.enter_context(tc.tile_pool(name="ps_vec", bufs=1, space="PSUM"))
    dram_scratch = ctx.enter_context(
        tc.tile_pool(name="dscratch", bufs=3 if has_bq else 2, space="DRAM")
    )
    # bias pool outside the stage-A region so chunk-0's biasT writes don't
    # WAR-wait on stage-A's hs/w tiles.
    biaspool = ctx.enter_context(tc.tile_pool(name="bias", bufs=3))

    # ---- stage A: G projection, w1, V ----
    with (
        tc.tile_pool(name="stage_a", bufs=1) as sa,
        tc.tile_pool(name="psum_a", bufs=3, space="PSUM") as psa,
    ):
        m_sb = sa.tile([P, HT, H], BF16)
        wv_sb = sa.tile([P, HT, H], BF16)
        # Critical startup wave, split across the two HWDGE issue queues:
        # Sync takes M's first column block then the remaining blocks;
        # Scalar takes hs chunk 0 as four 2-tile descriptors.  The G loop
        # (c outer, ot inner) consumes hs chunk 0 it-tile by it-tile as
        # the pieces land, then one fresh 1MB hs chunk per c-pass.
        # Sync streams all M column blocks (the ot-passes consume them in
        # order); Scalar streams hs chunk 0 as four 2-tile pieces (the
        # it-accumulation tracks the landing stream).
        for ob in range(HT):
            nc.sync.dma_start(
                out=m_sb[:, :, ob * P : (ob + 1) * P],
                in_=mT[:, :, ob * P : (ob + 1) * P],
            )
        for tt in range(0, HT, 2):
            nc.scalar.dma_start(
                out=hs_sb[:, tt : tt + 2, 0:CHUNK],
                in_=hsT[:, tt : tt + 2, 0:CHUNK],
            )
        # consts ride the scalar queue behind the critical wave
        nc.scalar.dma_start(out=m1_s, in_=ins["m1"])
        nc.scalar.dma_start(out=am_s, in_=ins["am"])
        if has_bv:
            nc.scalar.dma_start(out=bvB, in_=_bcast_ap(ins["bv"]))
        nc.vector.memset(ones_bf, 1.0)
        if has_bq:
            nc.scalar.dma_start(out=mb_s, in_=ins["mb"])
            nc.scalar.dma_start(out=c_s, in_=ins["c"])
        for c in range(1, NCH):
            nc.sync.dma_start(
                out=hs_sb[:, :, c * CHUNK : (c + 1) * CHUNK],
                in_=hsT[:, :, c * CHUNK : (c + 1) * CHUNK],
            )
        nc.sync.dma_start(out=wv_sb, in_=wvT)
        # masks for q-chunks 0/1 (chunks 2/3 stream in during stage B)
        for i in range(2):
            nc.sync.dma_start(
                out=mask_t[i], in_=relT[:, :, i * CHUNK : (i + 1) * CHUNK]
            )

        # Dummy matmuls: keep the PE busy (and the HAM clock-gate warm)
        # while the first m/hs tiles stream in. Zero data, never consumed.
        dummy_src = sa.tile([P, 256], BF16)
        nc.vector.memset(dummy_src, 0.0)
        dummy_ps = ps_vec.tile([P, 256], F32, tag="vec1", name="dummy_ps")
        for _ in range(NDUMMY):
            nc.tensor.matmul(
                dummy_ps,
                dummy_src[:, 0:P],
                dummy_src,
                start=True,
                stop=True,
                skip_group_check=True,
            )

        # GT[o, s] = sum_i M[i, o] * hsT[i, s].  c outer / ot inner: each
        # c-pass re-reads the already-resident M and consumes exactly one
        # fresh hs chunk, so the PE never outruns the startup DMA stream.
        # ot tiles run as interleaved pairs (two PSUM banks) except the
        # DMA-paced first pass — see emit_pv for the group-start rationale.
        for c in range(NCH):
            pair = 2 if c > 0 else 1
            for ot0 in range(0, HT, pair):
                ps_gs = [
                    psa.tile([P, CHUNK], F32, tag="pa", name="pa_g")
                    for _ in range(pair)
                ]
                for it in range(HT):
                    for k in range(pair):
                        ot = ot0 + k
                        nc.tensor.matmul(
                            ps_gs[k],
                            m_sb[:, it, ot * P : (ot + 1) * P],
                            hs_sb[:, it, c * CHUNK : (c + 1) * CHUNK],
                            start=(it == 0),
                            stop=(it == HT - 1),
                        )
                for k in range(pair):
                    ot = ot0 + k
                    tgt = (
                        GTb[:, ot, c * CHUNK : (c + 1) * CHUNK]
                        if ot < FP8_K0
                        else G8[:, ot - FP8_K0, c * CHUNK : (c + 1) * CHUNK]
                    )
                    nc.scalar.activation(tgt, ps_gs[k], AF.Identity, bias=0.0)

        # W1B precompute: w1[q]/32 = hs[q] . m1/32, all chunks, col-tiled 4x.
        # Emitted right after G so the DRAM-broadcast roundtrip completes
        # long before chunk 0 needs it.
        w1p = ps_vec.tile([P, CHUNK], F32, tag="vec1")
        for it in range(HT):
            for c in range(NCH):
                nc.tensor.matmul(
                    w1p[32 * c : 32 * c + 1, :],
                    m1_s[:, it : it + 1],
                    hs_sb[:, it, c * CHUNK : (c + 1) * CHUNK],
                    start=(it == 0),
                    stop=(it == HT - 1),
                    tile_position=(0, 32 * c),
                    skip_group_check=True,
                )
        w1rows = consts.tile([1, NCH, CHUNK], BF16)
        w1d = dram_scratch.tile([1, NCH, CHUNK], BF16)
        for c in range(NCH):
            if has_bq:
                # w1_full/32 = hs.m1/32 + (bq.d1)/32
                nc.scalar.activation(
                    w1rows[:, c, :],
                    w1p[32 * c : 32 * c + 1, :],
                    AF.Identity,
                    bias=c_s[0:1, 0:1],
                )
            else:
                nc.vector.tensor_copy(w1rows[:, c, :], w1p[32 * c : 32 * c + 1, :])
            nc.sync.dma_start(out=w1d[:, c, :], in_=w1rows[:, c, :])
            nc.sync.dma_start(out=W1B[:, c, :], in_=_bcast_ap(w1d[:, c, :]))

        if has_bq:
            # kb[k]/32 = hs[k] . (Wk.T bq)/32, added to the per-key exp bias.
            kbp = ps_vec.tile([P, CHUNK], F32, tag="vec1")
            for it in range(HT):
                for c in range(NCH):
                    nc.tensor.matmul(
                        kbp[32 * c : 32 * c + 1, :],
                        mb_s[:, it : it + 1],
                        hs_sb[:, it, c * CHUNK : (c + 1) * CHUNK],
                        start=(it == 0),
                        stop=(it == HT - 1),
                        tile_position=(0, 32 * c),
                        skip_group_check=True,
                    )
            kbrow = consts.tile([1, NCH, CHUNK], F32)
            for c in range(NCH):
                nc.vector.tensor_copy(kbrow[:, c, :], kbp[32 * c : 32 * c + 1, :])
            kbd = dram_scratch.tile([1, NCH, CHUNK], F32)
            nc.sync.dma_start(out=kbd, in_=kbrow)
            kb_s = consts.tile([P, JT], F32)
            nc.sync.dma_start(
                out=kb_s,
                in_=bass.AP(tensor=kbd.tensor, offset=kbd.offset, ap=[[1, P], [P, JT]]),
            )
            am_eff = consts.tile([P, JT], F32)
            nc.vector.tensor_tensor(am_eff, am_s, kb_s, op=ALU.add)
            am_x = am_eff
        else:
            am_x = am_s

        # fp8 copies of the hs k-tiles used by the DoubleRow score matmuls
        # (scalar engine; overlaps the V matmuls below)
        for i in range(NF8):
            nc.scalar.activation(
                hs8[:, i, :], hs_sb[:, i + FP8_K0, :], AF.Identity, bias=0.0
            )

        # V[j, o] = sum_i hsT[i, j] * WvT[i, o] + bv[o].  V last: it has no
        # chunk-0 consumers until PV, so its matmuls give the scheduler PE
        # filler while chunk-0's softmax pipeline warms up.
        for jt in range(JT):
            pss = [
                psa.tile([P, CHUNK], F32, tag="pa", name=f"pav_{i}")
                for i in range(HC)
            ]
            for it in range(HT):
                for oc in range(HC):
                    nc.tensor.matmul(
                        pss[oc],
                        hs_sb[:, it, jt * P : (jt + 1) * P],
                        wv_sb[:, it, oc * CHUNK : (oc + 1) * CHUNK],
                        start=(it == 0),
                        stop=(it == HT - 1),
                    )
            for oc in range(HC):
                if has_bv:
                    nc.vector.tensor_tensor(
                        V[:, jt, oc * CHUNK : (oc + 1) * CHUNK],
                        pss[oc],
                        bvB[:, oc * CHUNK : (oc + 1) * CHUNK],
                        op=ALU.add,
                    )
                else:
                    # scalar-engine copy keeps the vector queue clear for
                    # chunk 0's softmax pipeline
                    nc.scalar.activation(
                        V[:, jt, oc * CHUNK : (oc + 1) * CHUNK],
                        pss[oc],
                        AF.Identity,
                        bias=0.0,
                    )

    # ---- stage B pools ----
    epool = ctx.enter_context(tc.tile_pool(name="E", bufs=2))
    esumpool = ctx.enter_context(tc.tile_pool(name="esum", bufs=2))
    recpool = ctx.enter_context(tc.tile_pool(name="rec", bufs=2))
    outpool = ctx.enter_context(tc.tile_pool(name="out", bufs=3))
    ps_pv = ctx.enter_context(tc.tile_pool(name="ps_pv", bufs=4, space="PSUM"))

    # ---- stage B: per query chunk ----
    # PV for chunk c is emitted after the scores/softmax of chunk c+1, so
    # the denom->reciprocal->broadcast chain of chunk c overlaps an entire
    # scores phase and the PE never waits on it.
    deferred_pv = []

    def pv_epilogue(c, dt, w, h, ps2, recB):
        ot_t = outpool.tile([P, w], BF16)
        nc.vector.tensor_tensor(
            ot_t, ps2, recB[:, h * w : (h + 1) * w], op=ALU.mult
        )
        nc.sync.dma_start(
            out=ctxT[:, dt, c * CHUNK + h * w : c * CHUNK + (h + 1) * w],
            in_=ot_t,
        )

    def emit_pv(c, E, recB, last=False):
        # dt tiles in interleaved pairs: two PSUM banks accumulate at once,
        # so the group-start semaphore wait (which exposes LDWEIGHTS,
        # ~+160ns on the first matmul of every group) is paid once per
        # pair instead of once per group.
        for dt in range(0, HT, 2):
            if last and dt == HT - 2:
                break
            ps2a = ps_pv.tile([P, CHUNK], F32, tag="pv")
            ps2b = ps_pv.tile([P, CHUNK], F32, tag="pv")
            for jt in range(JT):
                for ps2, d in ((ps2a, dt), (ps2b, dt + 1)):
                    nc.tensor.matmul(
                        ps2,
                        V[:, jt, d * P : (d + 1) * P],
                        E[:, jt, :],
                        start=(jt == 0),
                        stop=(jt == JT - 1),
                    )
            pv_epilogue(c, dt, CHUNK, 0, ps2a, recB)
            pv_epilogue(c, dt + 1, CHUNK, 0, ps2b, recB)
        if last:
            # final two dt tiles unpaired, the very last split in half so
            # the trailing (non-overlapped) multiply+DMA epilogue is small.
            # (always full-bank [P, CHUNK] psum tiles — mixed shapes would
            # multiply the pool footprint — writing only the first w cols)
            for dt, nsplit in ((HT - 2, 1), (HT - 1, 2)):
                w = CHUNK // nsplit
                for h in range(nsplit):
                    ps2 = ps_pv.tile([P, CHUNK], F32, tag="pv")
                    for jt in range(JT):
                        nc.tensor.matmul(
                            ps2[:, 0:w],
                            V[:, jt, dt * P : (dt + 1) * P],
                            E[:, jt, h * w : (h + 1) * w],
                            start=(jt == 0),
                            stop=(jt == JT - 1),
                        )
                    pv_epilogue(c, dt, w, h, ps2[:, 0:w], recB)

    for c in range(NCH):
        cs = slice(c * CHUNK, (c + 1) * CHUNK)

        E = epool.tile([P, JT, CHUNK], BF16)
        # per-partition running sum of E on the (otherwise idle) GpSimd
        # engine, staggered one tile behind the exps.  The old 4x
        # col-tiled PE "denom quads" shared XBUSes with the DoubleRow
        # score matmuls (col-tiling needs 8 of 9, DR needs 2) and
        # stretched both; now the PE only sees two plain [128,1,512]
        # matmuls per chunk for the final cross-partition reduce.
        acc = esumpool.tile([P, CHUNK], F32, tag="acc")
        dps = ps_vec.tile([P, CHUNK], F32, tag="vec1")

        for jt in range(JT):
            biasT = biaspool.tile([P, CHUNK], BF16)
            nc.vector.tensor_tensor(
                biasT, mask_t[c % 2][:, jt, :], W1B[:, c, :], op=ALU.mult
            )
            ps = ps_score.tile([P, CHUNK], F32)
            for dt in range(FP8_K0):
                nc.tensor.matmul(
                    ps,
                    hs_sb[:, dt, jt * P : (jt + 1) * P],
                    GTb[:, dt, cs],
                    start=(dt == 0),
                    stop=False,
                )
            for i in range(0, NF8, 2):
                nc.tensor.matmul(
                    ps,
                    hs8[:, i : i + 2, jt * P : (jt + 1) * P],
                    G8[:, i : i + 2, cs],
                    start=(FP8_K0 == 0 and i == 0),
                    stop=(i == NF8 - 2),
                    perf_mode=DR,
                )
            nc.vector.scalar_tensor_tensor(
                ps, ps, SCALE, biasT, op0=ALU.mult, op1=ALU.add
            )
            nc.scalar.activation(E[:, jt, :], ps, AF.Exp, bias=am_x[:, jt : jt + 1])
            if jt == 0:
                nc.gpsimd.tensor_copy(acc, E[:, 0, :])
            else:
                nc.gpsimd.tensor_tensor(acc, acc, E[:, jt, :], op=ALU.add)

        # next-next chunk's mask reuses this chunk's buffer; issue the DMA
        # now (it WAR-waits on the last biasT read above, then has ~40us
        # to land 2MB before chunk c+2 reads it)
        if c + 2 < NCH:
            nc.sync.dma_start(
                out=mask_t[c % 2],
                in_=relT[:, :, (c + 2) * CHUNK : (c + 3) * CHUNK],
            )
        # acc -> bf16 hi/lo split (exact to ~2^-17) so the cross-partition
        # reduce runs as two cheap bf16 matmuls instead of one 1/4-rate
        # f32 one.  hi/lo on the (fast) DVE: the gpsimd adds have a whole
        # PV phase of slack, but hi/lo sit right before the PE's denom
        # matmuls and gpsimd's ~2us/op latency stalled the PE there.
        hi = esumpool.tile([P, CHUNK], BF16, tag="hi")
        nc.vector.tensor_copy(hi, acc)
        lo = esumpool.tile([P, CHUNK], BF16, tag="lo")
        nc.vector.tensor_tensor(lo, acc, hi, op=ALU.subtract)
        if deferred_pv:
            emit_pv(*deferred_pv.pop(0))
        nc.tensor.matmul(dps[0:1, :], ones_bf, hi, start=True, stop=False)
        nc.tensor.matmul(dps[0:1, :], ones_bf, lo, start=False, stop=True)

        # denom row -> DRAM -> partition-broadcast -> reciprocal
        dsum = recpool.tile([1, CHUNK], F32, tag="dsum")
        nc.vector.tensor_copy(dsum, dps[0:1, :])
        dsum_d = dram_scratch.tile([1, CHUNK], F32, tag="dsum_d")
        nc.sync.dma_start(out=dsum_d, in_=dsum)
        denB = recpool.tile([P, CHUNK], F32, tag="denB")
        nc.sync.dma_start(out=denB, in_=_bcast_ap(dsum_d))
        recB = recpool.tile([P, CHUNK], F32, tag="recB")
        rscr = recpool.tile([P, CHUNK], F32, tag="rscr")
        nc.vector.reciprocal_approx_accurate(recB, denB, rscr)

        deferred_pv.append((c, E, recB))
    while deferred_pv:
        last = len(deferred_pv) == 1
        emit_pv(*deferred_pv.pop(0), last=last)


def build_program(s=S, has_bq=False, has_bv=True):
    """Build + compile the per-core Bass program."""
    JT = s // P
    HT = H // P
    nc = bacc.Bacc("TRN2", target_bir_lowering=False, debug=False)
    ins = {
        "hsT": nc.dram_tensor("hsT", [H, s], BF16, kind="ExternalInput").ap(),
        "m": nc.dram_tensor("m", [H, H], BF16, kind="ExternalInput").ap(),
        "wvT": nc.dram_tensor("wvT", [H, H], BF16, kind="ExternalInput").ap(),
        "m1": nc.dram_tensor("m1", [P, HT], BF16, kind="ExternalInput").ap(),
        "am": nc.dram_tensor("am", [P, JT], F32, kind="ExternalInput").ap(),
        "relT": nc.dram_tensor("relT", [s, s], BF16, kind="ExternalInput").ap(),
    }
    if has_bv:
        ins["bv"] = nc.dram_tensor("bv", [1, H], BF16, kind="ExternalInput").ap()
    if has_bq:
        ins["mb"] = nc.dram_tensor("mb", [P, HT], BF16, kind="ExternalInput").ap()
        ins["c"] = nc.dram_tensor("c", [1, 1], F32, kind="ExternalInput").ap()
    outs = {
        "ctxT": nc.dram_tensor("ctxT", [H, s], BF16, kind="ExternalOutput").ap(),
    }
    with tile.TileContext(nc) as tc:
        _attn_kernel(tc, outs, ins, s=s, has_bq=has_bq, has_bv=has_bv)
    nc.compile()
    return nc


def make_in_maps(
    hidden_states,
    attention_mask,
    word_word_relation,
    Wq,
    bq,
    Wk,
    bk,
    Wv,
    bv,
    dist_emb,
    s=S,
):
    """Host-side sharding/layout marshalling: one batch per core.

    Weight-only folds (O(H^2), batch-independent): M = Wq.T @ Wk,
    m1 = Wq.T @ dist_emb[1].  bk only enters softmax-invariant terms.
    The relation tensor ships as the uint8 mask (rel == 1).
    """
    HT = H // P
    JT = s // P
    hs = np.asarray(hidden_states, dtype=np.float32)
    am = np.asarray(attention_mask, dtype=np.float32)
    rel = np.asarray(word_word_relation)
    maskb = (rel == 1).astype(NPBF16)
    Wqf = np.asarray(Wq, np.float32)
    Wkf = np.asarray(Wk, np.float32)
    Wvf = np.asarray(Wv, np.float32)
    d1 = np.asarray(dist_emb, np.float32)[1]
    m_h = np.ascontiguousarray((Wqf.T @ Wkf).astype(NPBF16))
    m1_h = np.ascontiguousarray(
        ((Wqf.T @ d1) * SCALE).reshape(HT, P).T.astype(NPBF16)
    )
    wvT = np.ascontiguousarray(Wvf.T.astype(NPBF16))
    bvf = np.asarray(bv, np.float32)
    has_bv = bool(np.any(bvf))
    if has_bv:
        bv_s = np.ascontiguousarray(bvf.astype(NPBF16).reshape(1, H))
    bqf = np.asarray(bq, np.float32)
    has_bq = bool(np.any(bqf))
    if has_bq:
        mb_h = np.ascontiguousarray(
            ((Wkf.T @ bqf) * SCALE).reshape(HT, P).T.astype(NPBF16)
        )
        c_h = np.ascontiguousarray(
            np.array([[float(bqf @ d1) * SCALE]], dtype=np.float32)
        )
    in_maps = []
    for b in range(hs.shape[0]):
        hsT = np.ascontiguousarray(hs[b].T.astype(NPBF16))
        relT = np.ascontiguousarray(maskb[b].T)
        am_s = np.ascontiguousarray(am[b, 0, 0].reshape(JT, P).T)
        im = {
            "hsT": hsT,
            "m": m_h,
            "wvT": wvT,
            "m1": m1_h,
            "am": am_s,
            "relT": relT,
        }
        if has_bv:
            im["bv"] = bv_s
        if has_bq:
            im["mb"] = mb_h
            im["c"] = c_h
        in_maps.append(im)
    return in_maps, has_bq, has_bv


_NC_CACHE = {}


def get_program(s=S, has_bq=False, has_bv=False):
    key = (s, has_bq, has_bv)
    if key not in _NC_CACHE:
        _NC_CACHE[key] = build_program(s, has_bq, has_bv)
    return _NC_CACHE[key]


def run(inputs: dict, trace: bool = False):
    """Run on hardware; returns (output [B,S,H] f32, BassKernelResults)."""
    in_maps, has_bq, has_bv = make_in_maps(**inputs)
    nc = get_program(S, has_bq, has_bv)
    res = run_bass_kernel_spmd(nc, in_maps, list(range(NCORES)), trace=trace)
    out = np.stack(
        [np.ascontiguousarray(r["ctxT"].T) for r in res.results], axis=0
    ).astype(np.float32)
    return out, res


def kernel(**inputs) -> np.ndarray:
    try:
        out, _ = run(inputs, trace=False)
    except Exception:
        # transient device/runtime hiccups have been observed once in a
        # while on back-to-back runs; one retry is cheap insurance
        out, _ = run(inputs, trace=False)
    return out
